# revision 23
# baseline (speedup 1.0000x reference)
"""Trainium2 Bass kernel for nn_DecoderModel (12-layer decoder w/ image token).

Sharding: Megatron TP-8.
  - qkv column-sharded (2 heads/core), proj row-sharded + AllReduce
  - fc column-sharded (512 dff/core), fc2 row-sharded + AllReduce
  - lm head vocab-sharded (host assembles shards; no gather collective)

Device layout: residual stream kept FEATURE-major (h^T: [D, T], D on
partitions, tokens on free axis).  LayerNorm is folded into the matmuls:
  y = x_hat @ W  with  x_hat = (x - mu) * r   (gamma folded into W host-side)
    = r .* (x @ W - mu * colsum(W))
The -mu*colsum(W) term is a rank-1 K=1 matmul accumulated into the same
PSUM; the r scaling rides on the PSUM->SBUF drain (DVE multiply with a
PE-broadcast r row).  Stats (sum, sum-of-squares) are computed with
ones-vector matmuls on the tensor engine.

Attention: scores computed kt-major (s^T[kt, qt]) so softmax is
exp + multiplicative causal mask; denominators come free via an
appended ones-column on the token-major V (built with PE transposes);
probabilities are normalized on the small o_aug output.

Matmuls run in float32r (TF32-like, 4x faster than fp32, ~1.5e-4 rel).
"""

import os
import numpy as np

from concourse import bacc, tile, mybir
from concourse import bass_utils

dt = mybir.dt
AF = mybir.ActivationFunctionType
ALU = mybir.AluOpType

# Model dims (hardcoded per contract)
B, S, D, H, L, V = 2, 512, 1024, 16, 12, 50257
HD = D // H          # 64
DFF = 4 * D          # 4096
T = B * S            # 1024 tokens
NC = 8               # cores
HL = H // NC         # 2 local heads
CW = HL * HD         # 128 cols per q/k/v shard
DFS = DFF // NC      # 512 dff shard
KT = 640             # padded kv length (5*128), real 513
NKC = KT // 128      # 5 kv chunks
VSH = 6283           # vocab rows per core (8*6283 = 50264 >= V)
VS = 6656            # padded vocab shard (13*512)
NVT = VS // 512      # 13 vocab tiles
EPS = 1e-5

F32 = dt.float32
F32R = dt.float32r
BF16 = dt.bfloat16


def _r(ap):
    return ap.bitcast(F32R)


def _build(nl, n_masks, has_bias_qkv, has_bias_proj, has_bias_fc, has_bias_fc2,
           has_bias_lm):
    nc = bacc.Bacc("TRN2", target_bir_lowering=False, debug=False,
                   num_devices=NC)

    dram = lambda n, sh, ty=F32, kind="ExternalInput": nc.dram_tensor(
        n, sh, ty, kind=kind).ap()

    h0T_d = dram("h0T", [D, T], F32R)
    wattn_d = dram("wattn", [nl, D, 3 * CW], F32R)
    csqkv_d = dram("csqkv", [nl, 1, 3 * CW], F32R)
    bqkv_d = dram("bqkv", [nl, 1, 3 * CW], F32R) if has_bias_qkv else None
    wproj_d = dram("wproj", [nl, CW, D], F32R)
    bproj_d = dram("bproj", [nl, 1, D], F32R) if has_bias_proj else None
    wfc_d = dram("wfc", [nl, D, DFS], F32R)
    csfc_d = dram("csfc", [nl, 1, DFS], F32R)
    bfc_d = dram("bfc", [nl, 1, DFS], F32R) if has_bias_fc else None
    wfc2_d = dram("wfc2", [nl, DFS, D], F32R)
    bfc2_d = dram("bfc2", [nl, 1, D], F32R) if has_bias_fc2 else None
    kivi_d = dram("kivi", [nl, 2, CW, B], F32R)
    mask_d = dram("mask", [n_masks, NKC, 128, S])
    ident_d = dram("ident", [128, 128], F32R)
    cones_d = dram("cones", [128, KT], F32R)  # cols 0:512 ones, rest zeros
    wteT_d = dram("wteT", [D, VS], F32R)
    blm_d = dram("blm", [1, VS], F32R) if has_bias_lm else None
    logits_d = dram("logits", [T, VS], kind="ExternalOutput")

    with tile.TileContext(nc) as tc:
        with (
            nc.allow_low_precision(reason="float32r matmul pipeline"),
            tc.tile_pool(name="const", bufs=1) as cpool,
            tc.tile_pool(name="resid", bufs=1) as hpool,
            tc.tile_pool(name="rows", bufs=2) as rpool,
            tc.tile_pool(name="dram", bufs=1, space="DRAM") as dpool,
        ):
            ident_sb = cpool.tile([128, 128], F32R, name="ident_sb")
            nc.sync.dma_start(ident_sb[:], ident_d[:])
            ones_col = cpool.tile([128, 1], F32R, name="ones_col")
            nc.sync.dma_start(ones_col[:], cones_d[:, 0:1])
            ones_row = cpool.tile([1, 512], F32R, name="ones_row")
            nc.sync.dma_start(ones_row[:], cones_d[0:1, 0:512])
            c_eps = cpool.tile([1, 1], F32, name="c_eps")
            nc.vector.memset(c_eps[:], EPS)
            c_invD = cpool.tile([1, 1], F32, name="c_invD")
            nc.vector.memset(c_invD[:], 1.0 / D)
            c_ninvD = cpool.tile([1, 1], F32, name="c_ninvD")
            nc.vector.memset(c_ninvD[:], -1.0 / D)

            mask_sb = []
            for b in range(n_masks):
                row = []
                for kc in range(NKC):
                    m = cpool.tile([128, S], F32, name=f"mask_{b}_{kc}")
                    nc.sync.dma_start(m[:], mask_d[b, kc])
                    row.append(m)
                mask_sb.append(row)
            mask_of = lambda b: mask_sb[min(b, n_masks - 1)]

            hT = []
            for kc in range(8):
                t_ = hpool.tile([128, T], F32R, name=f"hT{kc}")
                nc.sync.dma_start(t_[:], h0T_d[kc * 128:(kc + 1) * 128, :])
                hT.append(t_)

            def ln_stats(pfx, xsq_pool, ps_row, want_mur=False):
                """Returns (r_row [1,T], nm_row [1,T] = -mu, mur_row [1,T])."""
                r_row = rpool.tile([1, T], F32R, tag="r", name=f"r_{pfx}", bufs=1)
                nm_row = rpool.tile([1, T], F32R, tag="nm", name=f"nm_{pfx}", bufs=1)
                mur_row = (rpool.tile([1, T], F32R, tag="mur",
                                      name=f"mur_{pfx}")
                           if want_mur else None)
                for hf in range(2):
                    sl = slice(hf * 512, (hf + 1) * 512)
                    mu_ps = ps_row.tile([1, 512], F32, tag="mu", bufs=1)
                    for kc in range(8):
                        nc.tensor.matmul(mu_ps[:], ones_col[:],
                                         hT[kc][:, sl],
                                         start=(kc == 0), stop=(kc == 7))
                    ssq_ps = ps_row.tile([1, 512], F32, tag="ssq", bufs=1)
                    for kc in range(8):
                        xsq = xsq_pool.tile([128, 512], F32R, tag="xsq")
                        nc.scalar.activation(xsq[:], hT[kc][:, sl], AF.Square)
                        nc.tensor.matmul(ssq_ps[:], ones_col[:],
                                         xsq[:],
                                         start=(kc == 0), stop=(kc == 7))
                    musq = rpool.tile([1, 512], F32, tag="musq", bufs=1)
                    nc.scalar.activation(musq[:], mu_ps[:], AF.Square,
                                         scale=c_invD[:])
                    varr = rpool.tile([1, 512], F32, tag="varr", bufs=1)
                    nc.vector.scalar_tensor_tensor(
                        varr[:], ssq_ps[:], 1.0 / D, musq[:],
                        ALU.mult, ALU.subtract)
                    sd = rpool.tile([1, 512], F32, tag="sd", bufs=1)
                    nc.scalar.activation(sd[:], varr[:], AF.Sqrt, bias=c_eps[:])
                    nc.vector.reciprocal(r_row[:, sl], sd[:])
                    # nm = -mu  (mean * -1/D... mu_ps holds sum -> -sum/D)
                    nc.scalar.mul(nm_row[:, sl], mu_ps[:], c_ninvD[:])
                    if want_mur:
                        nc.vector.tensor_tensor(
                            mur_row[:, sl], nm_row[:, sl], r_row[:, sl],
                            ALU.mult)  # = -mu*r
                return r_row, nm_row, mur_row

            with (
                tc.tile_pool(name="wts", bufs=1) as wpool,
                tc.tile_pool(name="wts2", bufs=2) as wpool2,
                tc.tile_pool(name="act", bufs=1) as apool,
                tc.tile_pool(name="scratch", bufs=2) as spool,
                tc.tile_pool(name="ps_row", bufs=1, space="PSUM") as ps_row,
                tc.tile_pool(name="ps_bc", bufs=1, space="PSUM") as ps_bc,
                tc.tile_pool(name="ps_mm", bufs=4, space="PSUM") as ps_mm,
            ):
                for l in range(nl):
                    # ---- weights for this layer
                    wattn_sb = []
                    for kc in range(8):
                        w = wpool2.tile([128, 3 * CW], F32R, tag=f"wattn{kc}", bufs=1,
                                        name=f"wattn{kc}_{l}")
                        nc.sync.dma_start(
                            w[:], wattn_d[l, kc * 128:(kc + 1) * 128, :])
                        wattn_sb.append(w)
                    csqkv_sb = wpool2.tile([1, 3 * CW], F32R, tag="csqkv",
                                           name=f"csqkv_{l}")
                    nc.sync.dma_start(csqkv_sb[:], csqkv_d[l])
                    if has_bias_qkv:
                        bqkv_sb = wpool2.tile([1, 3 * CW], F32R, tag="bqkv",
                                              name=f"bqkv_{l}")
                        nc.sync.dma_start(bqkv_sb[:], bqkv_d[l])
                    wproj_sb = wpool.tile([128, D], F32R, tag="wproj",
                                          name=f"wproj_{l}")
                    nc.sync.dma_start(wproj_sb[:], wproj_d[l])
                    if has_bias_proj:
                        bproj_sb = wpool.tile([1, D], F32R, tag="bproj",
                                              name=f"bproj_{l}")
                        nc.sync.dma_start(bproj_sb[:], bproj_d[l])
                    wfc_sb = []
                    for kc in range(8):
                        w = wpool.tile([128, DFS], F32R, tag=f"wfc{kc}",
                                       name=f"wfc{kc}_{l}")
                        nc.sync.dma_start(
                            w[:], wfc_d[l, kc * 128:(kc + 1) * 128, :])
                        wfc_sb.append(w)
                    csfc_sb = wpool2.tile([1, DFS], F32R, tag="csfc",
                                          name=f"csfc_{l}")
                    nc.sync.dma_start(csfc_sb[:], csfc_d[l])
                    if has_bias_fc:
                        bfc_sb = wpool2.tile([1, DFS], F32R, tag="bfc",
                                             name=f"bfc_{l}")
                        nc.sync.dma_start(bfc_sb[:], bfc_d[l])
                    wfc2_sb = []
                    for kc in range(4):
                        w = wpool.tile([128, D], F32R, tag=f"wfc2{kc}",
                                       name=f"wfc2{kc}_{l}")
                        nc.sync.dma_start(
                            w[:], wfc2_d[l, kc * 128:(kc + 1) * 128, :])
                        wfc2_sb.append(w)
                    if has_bias_fc2:
                        bfc2_sb = wpool.tile([1, D], F32R, tag="bfc2",
                                             name=f"bfc2_{l}")
                        nc.sync.dma_start(bfc2_sb[:], bfc2_d[l])

                    # ---- LN1 stats
                    r1, nm1, _ = ln_stats(f"l{l}a", spool, ps_row)
                    rb1 = []
                    for hf in range(2):
                        bc = ps_bc.tile([128, 512], F32, tag="bc", bufs=1)
                        nc.tensor.matmul(bc[:], ones_row[:, 0:128],
                                         r1[:, hf * 512:(hf + 1) * 512],
                                         start=True, stop=True)
                        bcs = spool.tile([128, 512], F32, tag=f"rbs{hf}",
                                         bufs=1)
                        nc.scalar.copy(bcs[:], bc[:])
                        rb1.append(bcs)

                    # ---- QKV (cc: 0=q 1=k 2=v), halves are batches
                    q_sb = apool.tile([128, T], F32R, tag="q", name=f"q_{l}")
                    kT_sb, vT_sb = [], []
                    for b in range(B):
                        k_ = apool.tile([128, KT], F32R, tag=f"kT{b}",
                                        name=f"kT{b}_{l}")
                        v_ = apool.tile([128, KT], F32R, tag=f"vT{b}",
                                        name=f"vT{b}_{l}")
                        nc.sync.dma_start(k_[:, 0:1], kivi_d[l, 0, :, b:b + 1])
                        nc.sync.dma_start(v_[:, 0:1], kivi_d[l, 1, :, b:b + 1])
                        nc.sync.dma_start(k_[:, 513:KT], cones_d[:, 513:KT])
                        nc.sync.dma_start(v_[:, 513:KT], cones_d[:, 513:KT])
                        kT_sb.append(k_)
                        vT_sb.append(v_)

                    for cc in range(3):
                        csl = slice(cc * CW, (cc + 1) * CW)
                        for hf in range(2):
                            tsl = slice(hf * 512, (hf + 1) * 512)
                            ps = ps_mm.tile([128, 512], F32, tag="mm")
                            for kc in range(8):
                                nc.tensor.matmul(
                                    ps[:], wattn_sb[kc][:, csl],
                                    hT[kc][:, tsl],
                                    start=(kc == 0), stop=False)
                            last = not has_bias_qkv
                            nc.tensor.matmul(
                                ps[:], csqkv_sb[:, csl],
                                nm1[:, tsl], start=False, stop=last)
                            if has_bias_qkv:
                                nc.tensor.matmul(
                                    ps[:], bqkv_sb[:, csl],
                                    ones_row[:], start=False, stop=True)
                            if cc == 0:
                                out = q_sb[:, tsl]
                            elif cc == 1:
                                out = kT_sb[hf][:, 1:513]
                            else:
                                out = vT_sb[hf][:, 1:513]
                            nc.vector.tensor_tensor(out, ps[:], rb1[hf][:],
                                                    ALU.mult)

                    # ---- attention, per (batch, local head)
                    oT_sb = apool.tile([128, T], F32R, tag="oT", name=f"oT_{l}")
                    for b in range(B):
                        for h in range(HL):
                            hsl = slice(h * HD, (h + 1) * HD)
                            qsl = slice(b * 512, (b + 1) * 512)
                            p_tiles = []
                            for kc in range(NKC):
                                sps = ps_mm.tile([128, 512], F32, tag="mm")
                                nc.tensor.matmul(
                                    sps[:],
                                    kT_sb[b][hsl,
                                                kc * 128:(kc + 1) * 128],
                                    q_sb[hsl, qsl],
                                    start=True, stop=True)
                                e = spool.tile([128, 512], F32, tag="e")
                                nc.scalar.activation(e[:], sps[:], AF.Exp)
                                p = spool.tile([128, 512], F32R, tag=f"p{kc}", bufs=1)
                                nc.vector.tensor_tensor(
                                    p[:], e[:], mask_of(b)[kc][:], ALU.mult)
                                p_tiles.append(p)
                            # token-major V with ones column
                            vt_ps = ps_mm.tile([128, 512], F32R, tag="mm")
                            for kc in range(NKC):
                                nc.tensor.transpose(
                                    vt_ps[:, kc * 64:(kc + 1) * 64],
                                    vT_sb[b][hsl, kc * 128:(kc + 1) * 128],
                                    ident_sb[hsl, 0:HD])
                            v5 = spool.tile([128, NKC * 65], F32R, tag="v5", bufs=1)
                            v5v = v5.rearrange("p (c w) -> p c w", c=NKC)
                            vtv = vt_ps[:, 0:320].rearrange(
                                "p (c w) -> p c w", c=NKC)
                            nc.scalar.copy(v5v[:, :, 0:64], vtv[:])
                            nc.sync.dma_start(
                                v5v[:, :, 64:65],
                                cones_d[:, 0:NKC].unsqueeze(-1))
                            o_ps = ps_mm.tile([128, 512], F32, tag="mm")
                            for kc in range(NKC):
                                nc.tensor.matmul(
                                    o_ps[0:65, :],
                                    v5[:, kc * 65:(kc + 1) * 65],
                                    p_tiles[kc][:],
                                    start=(kc == 0), stop=(kc == NKC - 1))
                            rc = rpool.tile([1, 512], F32R, tag="rc")
                            nc.vector.reciprocal(rc[:], o_ps[64:65, :])
                            rbo = ps_mm.tile([128, 512], F32, tag="mm")
                            nc.tensor.matmul(rbo[0:64, :],
                                             ones_row[:, 0:64],
                                             rc[:], start=True, stop=True)
                            rbos = spool.tile([64, 512], F32, tag="rbos",
                                              bufs=1)
                            nc.scalar.copy(rbos[:], rbo[0:64, :])
                            nc.vector.tensor_tensor(
                                oT_sb[hsl, qsl], o_ps[0:HD, :],
                                rbos[:], ALU.mult)

                    # ---- proj (row shard) -> AR input
                    arin_a = dpool.tile([D, T], BF16, name=f"arin_a{l}")
                    arout_a = dpool.tile([D, T], BF16, name=f"arout_a{l}",
                                         addr_space="Shared")
                    for hf in range(2):
                        tsl = slice(hf * 512, (hf + 1) * 512)
                        for mc in range(8):
                            zps = ps_mm.tile([128, 512], F32, tag="mm")
                            last = not has_bias_proj
                            nc.tensor.matmul(
                                zps[:],
                                wproj_sb[:, mc * 128:(mc + 1) * 128],
                                oT_sb[:, tsl], start=True, stop=last)
                            if has_bias_proj:
                                nc.tensor.matmul(
                                    zps[:],
                                    bproj_sb[:, mc * 128:(mc + 1) * 128],
                                    ones_row[:], start=False, stop=True)
                            zsb = spool.tile([128, 512], BF16,
                                             tag="ardrain", bufs=2)
                            nc.scalar.copy(zsb[:], zps[:])
                            nc.sync.dma_start(
                                arin_a[mc * 128:(mc + 1) * 128, tsl], zsb[:])
                    nc.gpsimd.collective_compute(
                        "AllReduce", ALU.add,
                        replica_groups=[list(range(NC))],
                        ins=[arin_a.opt()], outs=[arout_a.opt()])
                    for kc in range(8):
                        z = spool.tile([128, T], BF16, tag="zz", bufs=1)
                        nc.sync.dma_start(
                            z[:], arout_a[kc * 128:(kc + 1) * 128, :])
                        nc.gpsimd.tensor_tensor(hT[kc][:], hT[kc][:], z[:],
                                                ALU.add)

                    # ---- LN2 + fc + gelu
                    r2, nm2, _ = ln_stats(f"l{l}b", spool, ps_row)
                    rb2 = []
                    for hf in range(2):
                        bc = ps_bc.tile([128, 512], F32, tag="bc", bufs=1)
                        nc.tensor.matmul(bc[:], ones_row[:, 0:128],
                                         r2[:, hf * 512:(hf + 1) * 512],
                                         start=True, stop=True)
                        bcs = spool.tile([128, 512], F32, tag=f"rbs{hf}",
                                         bufs=1)
                        nc.scalar.copy(bcs[:], bc[:])
                        rb2.append(bcs)
                    g_sb = []
                    for mc in range(4):
                        g = apool.tile([128, T], F32R, tag=f"g{mc}",
                                       name=f"g{mc}_{l}")
                        g_sb.append(g)
                    for mc in range(4):
                        csl = slice(mc * 128, (mc + 1) * 128)
                        for hf in range(2):
                            tsl = slice(hf * 512, (hf + 1) * 512)
                            ps = ps_mm.tile([128, 512], F32, tag="mm")
                            for kc in range(8):
                                nc.tensor.matmul(
                                    ps[:], wfc_sb[kc][:, csl],
                                    hT[kc][:, tsl],
                                    start=(kc == 0), stop=False)
                            last = not has_bias_fc
                            nc.tensor.matmul(
                                ps[:], csfc_sb[:, csl], nm2[:, tsl],
                                start=False, stop=last)
                            if has_bias_fc:
                                nc.tensor.matmul(
                                    ps[:], bfc_sb[:, csl],
                                    ones_row[:], start=False, stop=True)
                            pre = spool.tile([128, 512], F32, tag="pre", bufs=1)
                            nc.vector.tensor_tensor(pre[:], ps[:],
                                                    rb2[hf][:], ALU.mult)
                            nc.scalar.activation(g_sb[mc][:, tsl], pre[:],
                                                 AF.Gelu_apprx_tanh)

                    # ---- fc2 (row shard) -> AR
                    arin_m = dpool.tile([D, T], BF16, name=f"arin_m{l}")
                    arout_m = dpool.tile([D, T], BF16, name=f"arout_m{l}",
                                         addr_space="Shared")
                    for hf in range(2):
                        tsl = slice(hf * 512, (hf + 1) * 512)
                        for mc in range(8):
                            msl = slice(mc * 128, (mc + 1) * 128)
                            zps = ps_mm.tile([128, 512], F32, tag="mm")
                            for kc in range(4):
                                lastk = (kc == 3) and not has_bias_fc2
                                nc.tensor.matmul(
                                    zps[:], wfc2_sb[kc][:, msl],
                                    g_sb[kc][:, tsl],
                                    start=(kc == 0), stop=lastk)
                            if has_bias_fc2:
                                nc.tensor.matmul(
                                    zps[:], bfc2_sb[:, msl],
                                    ones_row[:], start=False, stop=True)
                            zsb = spool.tile([128, 512], BF16,
                                             tag="ardrain", bufs=2)
                            nc.vector.tensor_copy(zsb[:], zps[:])
                            nc.sync.dma_start(arin_m[msl, tsl], zsb[:])
                    nc.gpsimd.collective_compute(
                        "AllReduce", ALU.add,
                        replica_groups=[list(range(NC))],
                        ins=[arin_m.opt()], outs=[arout_m.opt()])
                    for kc in range(8):
                        z = spool.tile([128, T], BF16, tag="zz", bufs=1)
                        nc.sync.dma_start(
                            z[:], arout_m[kc * 128:(kc + 1) * 128, :])
                        nc.gpsimd.tensor_tensor(hT[kc][:], hT[kc][:], z[:],
                                                ALU.add)

            # ================= LM head =================
            with (
                tc.tile_pool(name="lm_w", bufs=2) as lwpool,
                tc.tile_pool(name="lm_x", bufs=1) as lxpool,
                tc.tile_pool(name="lm_sc", bufs=2) as lspool,
                tc.tile_pool(name="ps_lmrow", bufs=2, space="PSUM") as ps_lr,
                tc.tile_pool(name="ps_lmbc", bufs=1, space="PSUM") as ps_lbc,
                tc.tile_pool(name="ps_lm", bufs=4, space="PSUM") as ps_lm,
            ):
                if has_bias_lm:
                    blm_sb = lwpool.tile([1, VS], F32R, tag="blm",
                                         name="blm_sb", bufs=1)
                    nc.sync.dma_start(blm_sb[:], blm_d[:])
                rf, nmf, murf = ln_stats("lnf", lspool, ps_lr, want_mur=True)
                xf = []
                for kc in range(8):
                    x_ = lxpool.tile([128, T], F32R, tag=f"xf{kc}",
                                     name=f"xf{kc}")
                    xf.append(x_)
                for hf in range(2):
                    tsl = slice(hf * 512, (hf + 1) * 512)
                    rbf = ps_lbc.tile([128, 512], F32, tag="rbf")
                    nc.tensor.matmul(rbf[:], ones_row[:, 0:128],
                                     rf[:, tsl], start=True, stop=True)
                    mrb = ps_lbc.tile([128, 512], F32, tag="mrb")
                    nc.tensor.matmul(mrb[:], ones_row[:, 0:128],
                                     murf[:, tsl], start=True, stop=True)
                    for kc in range(8):
                        # xf = h*r + (-mu*r)  (mur row is -mu*r)
                        nc.vector.tensor_tensor(
                            xf[kc][:, tsl], hT[kc][:, tsl], rbf[:], ALU.mult)
                        nc.vector.tensor_tensor(
                            xf[kc][:, tsl], xf[kc][:, tsl], mrb[:], ALU.add)

                for vt in range(NVT):
                    vsl = slice(vt * 512, (vt + 1) * 512)
                    wt_sb = []
                    for kc in range(8):
                        w = lwpool.tile([128, 512], F32R, tag=f"wte{kc}",
                                        name=f"wte{kc}_{vt}")
                        nc.sync.dma_start(
                            w[:], wteT_d[kc * 128:(kc + 1) * 128, vsl])
                        wt_sb.append(w)
                    for tcc in range(8):
                        csl = slice(tcc * 128, (tcc + 1) * 128)
                        lg = ps_lm.tile([128, 512], F32, tag="lg")
                        for kc in range(8):
                            lastk = (kc == 7) and not has_bias_lm
                            nc.tensor.matmul(
                                lg[:], xf[kc][:, csl], wt_sb[kc][:],
                                start=(kc == 0), stop=lastk)
                        if has_bias_lm:
                            nc.tensor.matmul(
                                lg[:], ones_row[:, 0:128],
                                blm_sb[:, vsl],
                                start=False, stop=True)
                        lsb = lspool.tile([128, 512], F32, tag="lmdrain",
                                          bufs=4)
                        if tcc % 2 == 0:
                            nc.scalar.copy(lsb[:], lg[:])
                        else:
                            nc.vector.tensor_copy(lsb[:], lg[:])
                        nc.sync.dma_start(logits_d[csl, vsl], lsb[:])

    nc.compile()
    return nc


def _prep(inputs):
    """Host-side preprocessing. Returns (in_maps, meta)."""
    f = lambda x: np.asarray(x, dtype=np.float32)
    ids = np.asarray(inputs["input_ids"]).astype(np.int64)
    am = f(inputs["attention_mask"])
    ihs = f(inputs["image_hidden_states"])
    wte = f(inputs["wte"])
    ft_W1, ft_b1 = f(inputs["ft_W1"]), f(inputs["ft_b1"])
    ft_W2, ft_b2 = f(inputs["ft_W2"]), f(inputs["ft_b2"])
    ln1_g, ln1_b = f(inputs["ln1_g"]), f(inputs["ln1_b"])
    Wattn, battn = f(inputs["Wattn"]), f(inputs["battn"])
    Wuk, buk = f(inputs["Wuk"]), f(inputs["buk"])
    Wuv, buv = f(inputs["Wuv"]), f(inputs["buv"])
    Wproj, bproj = f(inputs["Wproj"]), f(inputs["bproj"])
    ln2_g, ln2_b = f(inputs["ln2_g"]), f(inputs["ln2_b"])
    Wfc, bfc = f(inputs["Wfc"]), f(inputs["bfc"])
    Wfc2, bfc2 = f(inputs["Wfc2"]), f(inputs["bfc2"])
    lnf_g, lnf_b = f(inputs["lnf_g"]), f(inputs["lnf_b"])

    nl = int(os.environ.get("BASS_NLAYERS", str(L)))

    # embedding + image transform
    h0 = wte[ids.reshape(-1)] + np.tile(wte[:S], (B, 1))  # [T, D]
    h0T = np.ascontiguousarray(h0.T)
    img = np.maximum(ihs @ ft_W1 + ft_b1, 0.0) @ ft_W2 + ft_b2  # [B, D]

    # image k/v for all layers: [nl, B, D]
    ki = np.einsum("bd,ldm->lbm", img, Wuk[:nl]) + buk[:nl][:, None, :]
    vi = np.einsum("bd,ldm->lbm", img, Wuv[:nl]) + buv[:nl][:, None, :]

    # causal multiplicative mask [B, NKC, 128, S]
    j = np.arange(KT)
    i = np.arange(S)
    causal = (j[:, None] <= i[None, :] + 1) & (j[:, None] <= 512)
    causal[0, :] = True
    mask = np.zeros((B, KT, S), np.float32)
    for b in range(B):
        m = causal.astype(np.float32).copy()
        amb = np.concatenate([[1.0], am[b], np.zeros(KT - S - 1, np.float32)])
        m *= amb[:, None]
        m[0, :] = 1.0  # image col always visible
        mask[b] = m
    mask = np.ascontiguousarray(mask.reshape(B, NKC, 128, S))
    if B == 2 and np.array_equal(mask[0], mask[1]):
        mask = mask[0:1]
    n_masks = mask.shape[0]

    # 2x2 tiling of eye(64): any 64-aligned [64,64] slice is identity
    ident = np.tile(np.eye(HD, dtype=np.float32), (2, 2))
    cones = np.zeros((128, KT), np.float32)
    cones[:, :512] = 1.0

    # scale for q
    qs = 1.0 / np.sqrt(np.float32(HD))

    in_maps = []
    bias_flags = None
    for c in range(NC):
        hg = [c * HL + t for t in range(HL)]
        qcols = np.concatenate([np.arange(h * HD, (h + 1) * HD) for h in hg])
        kcols = D + qcols
        vcols = 2 * D + qcols

        wq = Wattn[:nl][:, :, qcols] * qs
        wk = Wattn[:nl][:, :, kcols]
        wv = Wattn[:nl][:, :, vcols]
        wqkv = np.concatenate([wq, wk, wv], axis=2)  # [nl, D, 384]
        wqkv = ln1_g[:nl][:, :, None] * wqkv
        csqkv = wqkv.sum(axis=1, keepdims=True)  # [nl, 1, 384]
        bq = battn[:nl][:, qcols] * qs
        bk = battn[:nl][:, kcols]
        bv = battn[:nl][:, vcols]
        bqkv = np.concatenate([bq, bk, bv], axis=1)[:, None, :]
        bqkv = bqkv + np.einsum("ld,ldm->lm", ln1_b[:nl],
                                np.concatenate([Wattn[:nl][:, :, qcols] * qs,
                                                Wattn[:nl][:, :, kcols],
                                                Wattn[:nl][:, :, vcols]],
                                               axis=2))[:, None, :]

        rows = qcols  # proj rows for these heads
        wproj_c = np.ascontiguousarray(Wproj[:nl][:, rows, :])
        bproj_c = (bproj[:nl] / NC)[:, None, :]

        wfc_c = ln2_g[:nl][:, :, None] * Wfc[:nl][:, :,
                                                  c * DFS:(c + 1) * DFS]
        csfc_c = wfc_c.sum(axis=1, keepdims=True)
        bfc_c = (bfc[:nl][:, c * DFS:(c + 1) * DFS][:, None, :]
                 + np.einsum("ld,ldm->lm", ln2_b[:nl],
                             Wfc[:nl][:, :, c * DFS:(c + 1) * DFS])[:, None, :])
        wfc2_c = np.ascontiguousarray(Wfc2[:nl][:, c * DFS:(c + 1) * DFS, :])
        bfc2_c = (bfc2[:nl] / NC)[:, None, :]

        kivi = np.stack([
            np.ascontiguousarray(ki[:, :, qcols].transpose(0, 2, 1)),
            np.ascontiguousarray(vi[:, :, qcols].transpose(0, 2, 1)),
        ], axis=1)  # [nl, 2, 128, B]

        v0 = c * VSH
        v1 = min(V, v0 + VSH)
        wt_rows = wte[v0:v1] * lnf_g[None, :]  # [real, D]
        wteT_c = np.zeros((D, VS), np.float32)
        wteT_c[:, : v1 - v0] = wt_rows.T
        blm_row = lnf_b @ wte[v0:v1].T  # [real]
        blm_c = np.zeros((1, VS), np.float32)
        blm_c[0, : v1 - v0] = blm_row

        m = {
            "h0T": h0T, "wattn": np.ascontiguousarray(wqkv),
            "csqkv": np.ascontiguousarray(csqkv),
            "wproj": wproj_c,
            "wfc": np.ascontiguousarray(wfc_c),
            "csfc": np.ascontiguousarray(csfc_c),
            "wfc2": wfc2_c, "kivi": kivi, "mask": mask, "ident": ident,
            "wteT": wteT_c, "cones": cones,
        }
        m["_bqkv"] = np.ascontiguousarray(bqkv)
        m["_bproj"] = np.ascontiguousarray(bproj_c)
        m["_bfc"] = np.ascontiguousarray(bfc_c)
        m["_bfc2"] = np.ascontiguousarray(bfc2_c)
        m["_blm"] = blm_c
        in_maps.append(m)
    names = ("bqkv", "bproj", "bfc", "bfc2", "blm")
    bias_flags = tuple(
        bool(any(np.any(m["_" + n]) for m in in_maps)) for n in names)
    for m in in_maps:
        for n, flag in zip(names, bias_flags):
            arr = m.pop("_" + n)
            if flag:
                m[n] = arr
    return in_maps, nl, n_masks, bias_flags


_LAST_RESULTS = {}


def kernel(**inputs):
    in_maps, nl, n_masks, bias_flags = _prep(inputs)
    nc = _build(nl, n_masks, *bias_flags)
    trace = bool(int(os.environ.get("BASS_KERNEL_TRACE", "0")))
    res = bass_utils.run_bass_kernel_spmd(
        nc, in_maps, core_ids=list(range(NC)), trace=trace)
    _LAST_RESULTS["res"] = res
    logits = np.empty((T, V), np.float32)
    for c in range(NC):
        v0 = c * VSH
        v1 = min(V, v0 + VSH)
        logits[:, v0:v1] = res.results[c]["logits"][:, : v1 - v0]
    return logits.reshape(B, S, V)


# revision 24
# speedup vs baseline: 1.3275x; 1.3275x over previous
"""Trainium2 Bass kernel for nn_DecoderModel (12-layer decoder w/ image token).

Sharding: Megatron TP-8.
  - qkv column-sharded (2 heads/core), proj row-sharded + AllReduce
  - fc column-sharded (512 dff/core), fc2 row-sharded + AllReduce
  - lm head vocab-sharded (host assembles shards; no gather collective)

Device layout: residual stream kept FEATURE-major (h^T: [D, T], D on
partitions, tokens on free axis).  LayerNorm is folded into the matmuls:
  y = x_hat @ W  with  x_hat = (x - mu) * r   (gamma folded into W host-side)
    = r .* (x @ W - mu * colsum(W))
The -mu*colsum(W) term is a rank-1 K=1 matmul accumulated into the same
PSUM; the r scaling rides on the PSUM->SBUF drain (DVE multiply with a
PE-broadcast r row).  Stats (sum, sum-of-squares) are computed with
ones-vector matmuls on the tensor engine.

Attention: scores computed kt-major (s^T[kt, qt]) so softmax is
exp + multiplicative causal mask; denominators come free via an
appended ones-column on the token-major V (built with PE transposes);
probabilities are normalized on the small o_aug output.

Matmuls run in float32r (TF32-like, 4x faster than fp32, ~1.5e-4 rel).
"""

import os
import numpy as np

from concourse import bacc, tile, mybir
from concourse import bass_utils

dt = mybir.dt
AF = mybir.ActivationFunctionType
ALU = mybir.AluOpType

# Model dims (hardcoded per contract)
B, S, D, H, L, V = 2, 512, 1024, 16, 12, 50257
HD = D // H          # 64
DFF = 4 * D          # 4096
T = B * S            # 1024 tokens
NC = 8               # cores
HL = H // NC         # 2 local heads
CW = HL * HD         # 128 cols per q/k/v shard
DFS = DFF // NC      # 512 dff shard
KT = 640             # padded kv length (5*128), real 513
NKC = KT // 128      # 5 kv chunks
VSH = 6283           # vocab rows per core (8*6283 = 50264 >= V)
VS = 6656            # padded vocab shard (13*512)
NVT = VS // 512      # 13 vocab tiles
EPS = 1e-5

F32 = dt.float32
F32R = dt.float32r
BF16 = dt.bfloat16


def _r(ap):
    return ap.bitcast(F32R)


def _build(nl, n_masks, has_bias_qkv, has_bias_proj, has_bias_fc, has_bias_fc2,
           has_bias_lm):
    nc = bacc.Bacc("TRN2", target_bir_lowering=False, debug=False,
                   num_devices=NC)

    dram = lambda n, sh, ty=F32, kind="ExternalInput": nc.dram_tensor(
        n, sh, ty, kind=kind).ap()

    h0T_d = dram("h0T", [D, T], F32R)
    wattn_d = dram("wattn", [nl, D, 3 * CW], F32R)
    csqkv_d = dram("csqkv", [nl, 1, 3 * CW], F32R)
    bqkv_d = dram("bqkv", [nl, 1, 3 * CW], F32R) if has_bias_qkv else None
    wproj_d = dram("wproj", [nl, CW, D], F32R)
    bproj_d = dram("bproj", [nl, 1, D], F32R) if has_bias_proj else None
    wfc_d = dram("wfc", [nl, D, DFS], F32R)
    csfc_d = dram("csfc", [nl, 1, DFS], F32R)
    bfc_d = dram("bfc", [nl, 1, DFS], F32R) if has_bias_fc else None
    wfc2_d = dram("wfc2", [nl, DFS, D], F32R)
    bfc2_d = dram("bfc2", [nl, 1, D], F32R) if has_bias_fc2 else None
    kivi_d = dram("kivi", [nl, 2, CW, B], F32R)
    mask_d = dram("mask", [n_masks, NKC, 128, S])
    ident_d = dram("ident", [128, 128], F32R)
    cones_d = dram("cones", [128, KT], F32R)  # cols 0:512 ones, rest zeros
    wteT_d = dram("wteT", [D, VS], F32R)
    blm_d = dram("blm", [1, VS], F32R) if has_bias_lm else None
    logits_d = dram("logits", [T, VS], kind="ExternalOutput")

    with tile.TileContext(nc) as tc:
        with (
            nc.allow_low_precision(reason="float32r matmul pipeline"),
            tc.tile_pool(name="const", bufs=1) as cpool,
            tc.tile_pool(name="resid", bufs=1) as hpool,
            tc.tile_pool(name="rows", bufs=2) as rpool,
            tc.tile_pool(name="dram", bufs=1, space="DRAM") as dpool,
        ):
            ident_sb = cpool.tile([128, 128], F32R, name="ident_sb")
            nc.sync.dma_start(ident_sb[:], ident_d[:])
            ones_col = cpool.tile([128, 1], F32R, name="ones_col")
            nc.sync.dma_start(ones_col[:], cones_d[:, 0:1])
            ones_row = cpool.tile([1, 512], F32R, name="ones_row")
            nc.sync.dma_start(ones_row[:], cones_d[0:1, 0:512])
            c_eps = cpool.tile([1, 1], F32, name="c_eps")
            nc.vector.memset(c_eps[:], EPS)
            c_invD = cpool.tile([1, 1], F32, name="c_invD")
            nc.vector.memset(c_invD[:], 1.0 / D)
            c_ninvD = cpool.tile([1, 1], F32, name="c_ninvD")
            nc.vector.memset(c_ninvD[:], -1.0 / D)

            mask_sb = []
            for b in range(n_masks):
                row = []
                for kc in range(NKC):
                    m = cpool.tile([128, S], F32, name=f"mask_{b}_{kc}")
                    nc.sync.dma_start(m[:], mask_d[b, kc])
                    row.append(m)
                mask_sb.append(row)
            mask_of = lambda b: mask_sb[min(b, n_masks - 1)]

            # residual, split per (feature chunk, batch half)
            hT = []
            for kc in range(8):
                pair = []
                for hf in range(2):
                    t_ = hpool.tile([128, 512], F32R, name=f"hT{kc}_{hf}")
                    nc.sync.dma_start(
                        t_[:], h0T_d[kc * 128:(kc + 1) * 128,
                                     hf * 512:(hf + 1) * 512])
                    pair.append(t_)
                hT.append(pair)

            def ln_stats(pfx, xsq_pool, ps_row, want_mur=False):
                """Per-half rows: (r[hf] [1,512], nm[hf] = -mu, mur[hf])."""
                rs, nms, murs = [], [], []
                for hf in range(2):
                    r_row = rpool.tile([1, 512], F32R, tag=f"r{hf}",
                                       name=f"r_{pfx}{hf}", bufs=1)
                    nm_row = rpool.tile([1, 512], F32R, tag=f"nm{hf}",
                                        name=f"nm_{pfx}{hf}", bufs=1)
                    mu_ps = ps_row.tile([1, 512], F32, tag="mu", bufs=1)
                    for kc in range(8):
                        nc.tensor.matmul(mu_ps[:], ones_col[:],
                                         hT[kc][hf][:],
                                         start=(kc == 0), stop=(kc == 7))
                    ssq_ps = ps_row.tile([1, 512], F32, tag="ssq", bufs=1)
                    for kc in range(8):
                        xsq = xsq_pool.tile([128, 512], F32R, tag="xsq")
                        nc.scalar.activation(xsq[:], hT[kc][hf][:], AF.Square)
                        nc.tensor.matmul(ssq_ps[:], ones_col[:], xsq[:],
                                         start=(kc == 0), stop=(kc == 7))
                    musq = rpool.tile([1, 512], F32, tag="musq", bufs=1)
                    nc.scalar.activation(musq[:], mu_ps[:], AF.Square,
                                         scale=c_invD[:])
                    varr = rpool.tile([1, 512], F32, tag="varr", bufs=1)
                    nc.vector.scalar_tensor_tensor(
                        varr[:], ssq_ps[:], 1.0 / D, musq[:],
                        ALU.mult, ALU.subtract)
                    sd = rpool.tile([1, 512], F32, tag="sd", bufs=1)
                    nc.scalar.activation(sd[:], varr[:], AF.Sqrt,
                                         bias=c_eps[:])
                    nc.vector.reciprocal(r_row[:], sd[:])
                    nc.scalar.mul(nm_row[:], mu_ps[:], c_ninvD[:])
                    rs.append(r_row)
                    nms.append(nm_row)
                    if want_mur:
                        mur_row = rpool.tile([1, 512], F32R, tag=f"mur{hf}",
                                             name=f"mur_{pfx}{hf}", bufs=1)
                        nc.vector.tensor_tensor(mur_row[:], nm_row[:],
                                                r_row[:], ALU.mult)
                        murs.append(mur_row)
                return rs, nms, murs

            def bcast(r_row, tag):
                """r row [1,512] -> SBUF [128,512] via K=1 matmul + copy."""
                bc = ps_bc.tile([128, 512], F32, tag="bc", bufs=1)
                nc.tensor.matmul(bc[:], ones_row[:, 0:128], r_row[:],
                                 start=True, stop=True)
                bcs = spool.tile([128, 512], F32, tag=tag, bufs=1)
                nc.scalar.copy(bcs[:], bc[:])
                return bcs

            with (
                tc.tile_pool(name="wts", bufs=1) as wpool,
                tc.tile_pool(name="wts2", bufs=2) as wpool2,
                tc.tile_pool(name="act", bufs=1) as apool,
                tc.tile_pool(name="scratch", bufs=2) as spool,
                tc.tile_pool(name="ps_row", bufs=1, space="PSUM") as ps_row,
                tc.tile_pool(name="ps_bc", bufs=1, space="PSUM") as ps_bc,
                tc.tile_pool(name="ps_mm", bufs=4, space="PSUM") as ps_mm,
            ):
                for l in range(nl):
                    # ---- weights for this layer
                    wattn_sb = []
                    for kc in range(8):
                        w = wpool2.tile([128, 3 * CW], F32R,
                                        tag=f"wattn{kc}", bufs=1,
                                        name=f"wattn{kc}_{l}")
                        nc.sync.dma_start(
                            w[:], wattn_d[l, kc * 128:(kc + 1) * 128, :])
                        wattn_sb.append(w)
                    csqkv_sb = wpool2.tile([1, 3 * CW], F32R, tag="csqkv",
                                           name=f"csqkv_{l}")
                    nc.sync.dma_start(csqkv_sb[:], csqkv_d[l])
                    if has_bias_qkv:
                        bqkv_sb = wpool2.tile([1, 3 * CW], F32R, tag="bqkv",
                                              name=f"bqkv_{l}")
                        nc.sync.dma_start(bqkv_sb[:], bqkv_d[l])
                    wproj_sb = wpool.tile([128, D], F32R, tag="wproj",
                                          name=f"wproj_{l}")
                    nc.sync.dma_start(wproj_sb[:], wproj_d[l])
                    if has_bias_proj:
                        bproj_sb = wpool.tile([1, D], F32R, tag="bproj",
                                              name=f"bproj_{l}")
                        nc.sync.dma_start(bproj_sb[:], bproj_d[l])
                    wfc_sb = []
                    for kc in range(8):
                        w = wpool.tile([128, DFS], F32R, tag=f"wfc{kc}",
                                       name=f"wfc{kc}_{l}")
                        nc.sync.dma_start(
                            w[:], wfc_d[l, kc * 128:(kc + 1) * 128, :])
                        wfc_sb.append(w)
                    csfc_sb = wpool2.tile([1, DFS], F32R, tag="csfc",
                                          name=f"csfc_{l}")
                    nc.sync.dma_start(csfc_sb[:], csfc_d[l])
                    if has_bias_fc:
                        bfc_sb = wpool2.tile([1, DFS], F32R, tag="bfc",
                                             name=f"bfc_{l}")
                        nc.sync.dma_start(bfc_sb[:], bfc_d[l])
                    wfc2_sb = []
                    for kc in range(4):
                        w = wpool.tile([128, D], F32R, tag=f"wfc2{kc}",
                                       name=f"wfc2{kc}_{l}")
                        nc.sync.dma_start(
                            w[:], wfc2_d[l, kc * 128:(kc + 1) * 128, :])
                        wfc2_sb.append(w)
                    if has_bias_fc2:
                        bfc2_sb = wpool.tile([1, D], F32R, tag="bfc2",
                                             name=f"bfc2_{l}")
                        nc.sync.dma_start(bfc2_sb[:], bfc2_d[l])

                    # ---- LN1 + QKV + attention + proj + AR, per half
                    r1, nm1, _ = ln_stats(f"l{l}a", spool, ps_row)
                    rb1 = [bcast(r1[hf], f"rbs{hf}") for hf in range(2)]

                    q_sb, kT_sb, vT_sb, oT_sb = [], [], [], []
                    for b in range(B):
                        qq = apool.tile([128, 512], F32R, tag=f"q{b}",
                                        name=f"q{b}_{l}")
                        k_ = apool.tile([128, KT], F32R, tag=f"kT{b}",
                                        name=f"kT{b}_{l}")
                        v_ = apool.tile([128, KT], F32R, tag=f"vT{b}",
                                        name=f"vT{b}_{l}")
                        nc.sync.dma_start(k_[:, 0:1], kivi_d[l, 0, :, b:b + 1])
                        nc.sync.dma_start(v_[:, 0:1], kivi_d[l, 1, :, b:b + 1])
                        nc.sync.dma_start(k_[:, 513:KT], cones_d[:, 513:KT])
                        nc.sync.dma_start(v_[:, 513:KT], cones_d[:, 513:KT])
                        oo = apool.tile([128, 512], F32R, tag=f"oT{b}",
                                        name=f"oT{b}_{l}")
                        q_sb.append(qq)
                        kT_sb.append(k_)
                        vT_sb.append(v_)
                        oT_sb.append(oo)

                    arin_a, arout_a = [], []
                    for hf in range(2):
                        arin_a.append(dpool.tile([D, 512], BF16,
                                                 name=f"arin_a{l}_{hf}"))
                        arout_a.append(dpool.tile([D, 512], BF16,
                                                  name=f"arout_a{l}_{hf}",
                                                  addr_space="Shared"))

                    for hf in range(2):
                        # qkv for this half
                        for cc in range(3):
                            csl = slice(cc * CW, (cc + 1) * CW)
                            ps = ps_mm.tile([128, 512], F32, tag="mm")
                            for kc in range(8):
                                nc.tensor.matmul(
                                    ps[:], wattn_sb[kc][:, csl],
                                    hT[kc][hf][:],
                                    start=(kc == 0), stop=False)
                            last = not has_bias_qkv
                            nc.tensor.matmul(
                                ps[:], csqkv_sb[:, csl], nm1[hf][:],
                                start=False, stop=last)
                            if has_bias_qkv:
                                nc.tensor.matmul(
                                    ps[:], bqkv_sb[:, csl], ones_row[:],
                                    start=False, stop=True)
                            if cc == 0:
                                out = q_sb[hf][:]
                            elif cc == 1:
                                out = kT_sb[hf][:, 1:513]
                            else:
                                out = vT_sb[hf][:, 1:513]
                            nc.vector.tensor_tensor(out, ps[:], rb1[hf][:],
                                                    ALU.mult)
                        # attention (batch == half)
                        b = hf
                        for h in range(HL):
                            hsl = slice(h * HD, (h + 1) * HD)
                            p_tiles = []
                            for kc in range(NKC):
                                sps = ps_mm.tile([128, 512], F32, tag="mm")
                                nc.tensor.matmul(
                                    sps[:],
                                    kT_sb[b][hsl,
                                             kc * 128:(kc + 1) * 128],
                                    q_sb[b][hsl, :],
                                    start=True, stop=True)
                                e = spool.tile([128, 512], F32, tag="e")
                                nc.scalar.activation(e[:], sps[:], AF.Exp)
                                p = spool.tile([128, 512], F32R,
                                               tag=f"p{kc}", bufs=1)
                                nc.vector.tensor_tensor(
                                    p[:], e[:], mask_of(b)[kc][:], ALU.mult)
                                p_tiles.append(p)
                            vt_ps = ps_mm.tile([128, 512], F32R, tag="mm")
                            for kc in range(NKC):
                                nc.tensor.transpose(
                                    vt_ps[:, kc * 64:(kc + 1) * 64],
                                    vT_sb[b][hsl, kc * 128:(kc + 1) * 128],
                                    ident_sb[hsl, 0:HD])
                            v5 = spool.tile([128, NKC * 65], F32R, tag="v5",
                                            bufs=1)
                            v5v = v5.rearrange("p (c w) -> p c w", c=NKC)
                            vtv = vt_ps[:, 0:320].rearrange(
                                "p (c w) -> p c w", c=NKC)
                            nc.scalar.copy(v5v[:, :, 0:64], vtv[:])
                            nc.sync.dma_start(
                                v5v[:, :, 64:65],
                                cones_d[:, 0:NKC].unsqueeze(-1))
                            o_ps = ps_mm.tile([128, 512], F32, tag="mm")
                            for kc in range(NKC):
                                nc.tensor.matmul(
                                    o_ps[0:65, :],
                                    v5[:, kc * 65:(kc + 1) * 65],
                                    p_tiles[kc][:],
                                    start=(kc == 0), stop=(kc == NKC - 1))
                            rc = rpool.tile([1, 512], F32R, tag="rc")
                            nc.vector.reciprocal(rc[:], o_ps[64:65, :])
                            rbo = ps_mm.tile([128, 512], F32, tag="mm")
                            nc.tensor.matmul(rbo[0:64, :],
                                             ones_row[:, 0:64],
                                             rc[:], start=True, stop=True)
                            rbos = spool.tile([64, 512], F32, tag="rbos",
                                              bufs=1)
                            nc.scalar.copy(rbos[:], rbo[0:64, :])
                            nc.vector.tensor_tensor(
                                oT_sb[b][hsl, :], o_ps[0:HD, :],
                                rbos[:], ALU.mult)
                        # proj partial -> AR input
                        for mc in range(8):
                            zps = ps_mm.tile([128, 512], F32, tag="mm")
                            last = not has_bias_proj
                            nc.tensor.matmul(
                                zps[:],
                                wproj_sb[:, mc * 128:(mc + 1) * 128],
                                oT_sb[hf][:], start=True, stop=last)
                            if has_bias_proj:
                                nc.tensor.matmul(
                                    zps[:],
                                    bproj_sb[:, mc * 128:(mc + 1) * 128],
                                    ones_row[:], start=False, stop=True)
                            zsb = spool.tile([128, 512], BF16,
                                             tag="ardrain", bufs=2)
                            nc.scalar.copy(zsb[:], zps[:])
                            nc.sync.dma_start(
                                arin_a[hf][mc * 128:(mc + 1) * 128, :],
                                zsb[:])
                        nc.gpsimd.collective_compute(
                            "AllReduce", ALU.add,
                            replica_groups=[list(range(NC))],
                            ins=[arin_a[hf].opt()], outs=[arout_a[hf].opt()])

                    # residual add (attn)
                    for hf in range(2):
                        for kc in range(8):
                            z = spool.tile([128, 512], BF16, tag="zz",
                                           bufs=2)
                            nc.sync.dma_start(
                                z[:], arout_a[hf][kc * 128:(kc + 1) * 128, :])
                            nc.gpsimd.tensor_tensor(hT[kc][hf][:],
                                                    hT[kc][hf][:], z[:],
                                                    ALU.add)

                    # ---- LN2 + fc + gelu + fc2 + AR, per half
                    r2, nm2, _ = ln_stats(f"l{l}b", spool, ps_row)
                    rb2 = [bcast(r2[hf], f"rbs{hf}") for hf in range(2)]
                    g_sb = [[apool.tile([128, 512], F32R, tag=f"g{mc}_{hf}",
                                        name=f"g{mc}_{hf}_{l}")
                             for hf in range(2)] for mc in range(4)]
                    arin_m, arout_m = [], []
                    for hf in range(2):
                        arin_m.append(dpool.tile([D, 512], BF16,
                                                 name=f"arin_m{l}_{hf}"))
                        arout_m.append(dpool.tile([D, 512], BF16,
                                                  name=f"arout_m{l}_{hf}",
                                                  addr_space="Shared"))
                    for hf in range(2):
                        for mc in range(4):
                            csl = slice(mc * 128, (mc + 1) * 128)
                            ps = ps_mm.tile([128, 512], F32, tag="mm")
                            for kc in range(8):
                                nc.tensor.matmul(
                                    ps[:], wfc_sb[kc][:, csl],
                                    hT[kc][hf][:],
                                    start=(kc == 0), stop=False)
                            last = not has_bias_fc
                            nc.tensor.matmul(
                                ps[:], csfc_sb[:, csl], nm2[hf][:],
                                start=False, stop=last)
                            if has_bias_fc:
                                nc.tensor.matmul(
                                    ps[:], bfc_sb[:, csl], ones_row[:],
                                    start=False, stop=True)
                            pre = spool.tile([128, 512], F32, tag="pre",
                                             bufs=1)
                            nc.vector.tensor_tensor(pre[:], ps[:],
                                                    rb2[hf][:], ALU.mult)
                            nc.scalar.activation(g_sb[mc][hf][:], pre[:],
                                                 AF.Gelu_apprx_tanh)
                        for mc in range(8):
                            msl = slice(mc * 128, (mc + 1) * 128)
                            zps = ps_mm.tile([128, 512], F32, tag="mm")
                            for kc in range(4):
                                lastk = (kc == 3) and not has_bias_fc2
                                nc.tensor.matmul(
                                    zps[:], wfc2_sb[kc][:, msl],
                                    g_sb[kc][hf][:],
                                    start=(kc == 0), stop=lastk)
                            if has_bias_fc2:
                                nc.tensor.matmul(
                                    zps[:], bfc2_sb[:, msl],
                                    ones_row[:], start=False, stop=True)
                            zsb = spool.tile([128, 512], BF16,
                                             tag="ardrain", bufs=2)
                            nc.vector.tensor_copy(zsb[:], zps[:])
                            nc.sync.dma_start(arin_m[hf][msl, :], zsb[:])
                        nc.gpsimd.collective_compute(
                            "AllReduce", ALU.add,
                            replica_groups=[list(range(NC))],
                            ins=[arin_m[hf].opt()], outs=[arout_m[hf].opt()])
                    for hf in range(2):
                        for kc in range(8):
                            z = spool.tile([128, 512], BF16, tag="zz",
                                           bufs=2)
                            nc.sync.dma_start(
                                z[:], arout_m[hf][kc * 128:(kc + 1) * 128, :])
                            nc.gpsimd.tensor_tensor(hT[kc][hf][:],
                                                    hT[kc][hf][:], z[:],
                                                    ALU.add)

            # ================= LM head =================
            with (
                tc.tile_pool(name="lm_w", bufs=2) as lwpool,
                tc.tile_pool(name="lm_x", bufs=1) as lxpool,
                tc.tile_pool(name="lm_sc", bufs=2) as lspool,
                tc.tile_pool(name="ps_lmrow", bufs=2, space="PSUM") as ps_lr,
                tc.tile_pool(name="ps_lmbc", bufs=1, space="PSUM") as ps_lbc,
                tc.tile_pool(name="ps_lm", bufs=4, space="PSUM") as ps_lm,
            ):
                if has_bias_lm:
                    blm_sb = lwpool.tile([1, VS], F32R, tag="blm",
                                         name="blm_sb", bufs=1)
                    nc.sync.dma_start(blm_sb[:], blm_d[:])
                rf, nmf, murf = ln_stats("lnf", lspool, ps_lr, want_mur=True)
                xf = []
                for kc in range(8):
                    x_ = lxpool.tile([128, T], F32R, tag=f"xf{kc}",
                                     name=f"xf{kc}")
                    xf.append(x_)
                for hf in range(2):
                    tsl = slice(hf * 512, (hf + 1) * 512)
                    rbf = ps_lbc.tile([128, 512], F32, tag="rbf")
                    nc.tensor.matmul(rbf[:], ones_row[:, 0:128],
                                     rf[hf][:], start=True, stop=True)
                    mrb = ps_lbc.tile([128, 512], F32, tag="mrb")
                    nc.tensor.matmul(mrb[:], ones_row[:, 0:128],
                                     murf[hf][:], start=True, stop=True)
                    for kc in range(8):
                        # xf = h*r + (-mu*r)
                        nc.vector.tensor_tensor(
                            xf[kc][:, tsl], hT[kc][hf][:], rbf[:], ALU.mult)
                        nc.vector.tensor_tensor(
                            xf[kc][:, tsl], xf[kc][:, tsl], mrb[:], ALU.add)

                for vt in range(NVT):
                    vsl = slice(vt * 512, (vt + 1) * 512)
                    wt_sb = []
                    for kc in range(8):
                        w = lwpool.tile([128, 512], F32R, tag=f"wte{kc}",
                                        name=f"wte{kc}_{vt}")
                        nc.sync.dma_start(
                            w[:], wteT_d[kc * 128:(kc + 1) * 128, vsl])
                        wt_sb.append(w)
                    for tcc in range(8):
                        csl = slice(tcc * 128, (tcc + 1) * 128)
                        lg = ps_lm.tile([128, 512], F32, tag="lg")
                        for kc in range(8):
                            lastk = (kc == 7) and not has_bias_lm
                            nc.tensor.matmul(
                                lg[:], xf[kc][:, csl], wt_sb[kc][:],
                                start=(kc == 0), stop=lastk)
                        if has_bias_lm:
                            nc.tensor.matmul(
                                lg[:], ones_row[:, 0:128],
                                blm_sb[:, vsl],
                                start=False, stop=True)
                        lsb = lspool.tile([128, 512], F32, tag="lmdrain",
                                          bufs=4)
                        if tcc % 2 == 0:
                            nc.scalar.copy(lsb[:], lg[:])
                        else:
                            nc.vector.tensor_copy(lsb[:], lg[:])
                        nc.sync.dma_start(logits_d[csl, vsl], lsb[:])

    nc.compile()
    return nc


def _prep(inputs):
    """Host-side preprocessing. Returns (in_maps, meta)."""
    f = lambda x: np.asarray(x, dtype=np.float32)
    ids = np.asarray(inputs["input_ids"]).astype(np.int64)
    am = f(inputs["attention_mask"])
    ihs = f(inputs["image_hidden_states"])
    wte = f(inputs["wte"])
    ft_W1, ft_b1 = f(inputs["ft_W1"]), f(inputs["ft_b1"])
    ft_W2, ft_b2 = f(inputs["ft_W2"]), f(inputs["ft_b2"])
    ln1_g, ln1_b = f(inputs["ln1_g"]), f(inputs["ln1_b"])
    Wattn, battn = f(inputs["Wattn"]), f(inputs["battn"])
    Wuk, buk = f(inputs["Wuk"]), f(inputs["buk"])
    Wuv, buv = f(inputs["Wuv"]), f(inputs["buv"])
    Wproj, bproj = f(inputs["Wproj"]), f(inputs["bproj"])
    ln2_g, ln2_b = f(inputs["ln2_g"]), f(inputs["ln2_b"])
    Wfc, bfc = f(inputs["Wfc"]), f(inputs["bfc"])
    Wfc2, bfc2 = f(inputs["Wfc2"]), f(inputs["bfc2"])
    lnf_g, lnf_b = f(inputs["lnf_g"]), f(inputs["lnf_b"])

    nl = int(os.environ.get("BASS_NLAYERS", str(L)))

    # embedding + image transform
    h0 = wte[ids.reshape(-1)] + np.tile(wte[:S], (B, 1))  # [T, D]
    h0T = np.ascontiguousarray(h0.T)
    img = np.maximum(ihs @ ft_W1 + ft_b1, 0.0) @ ft_W2 + ft_b2  # [B, D]

    # image k/v for all layers: [nl, B, D]
    ki = np.einsum("bd,ldm->lbm", img, Wuk[:nl]) + buk[:nl][:, None, :]
    vi = np.einsum("bd,ldm->lbm", img, Wuv[:nl]) + buv[:nl][:, None, :]

    # causal multiplicative mask [B, NKC, 128, S]
    j = np.arange(KT)
    i = np.arange(S)
    causal = (j[:, None] <= i[None, :] + 1) & (j[:, None] <= 512)
    causal[0, :] = True
    mask = np.zeros((B, KT, S), np.float32)
    for b in range(B):
        m = causal.astype(np.float32).copy()
        amb = np.concatenate([[1.0], am[b], np.zeros(KT - S - 1, np.float32)])
        m *= amb[:, None]
        m[0, :] = 1.0  # image col always visible
        mask[b] = m
    mask = np.ascontiguousarray(mask.reshape(B, NKC, 128, S))
    if B == 2 and np.array_equal(mask[0], mask[1]):
        mask = mask[0:1]
    n_masks = mask.shape[0]

    # 2x2 tiling of eye(64): any 64-aligned [64,64] slice is identity
    ident = np.tile(np.eye(HD, dtype=np.float32), (2, 2))
    cones = np.zeros((128, KT), np.float32)
    cones[:, :512] = 1.0

    # scale for q
    qs = 1.0 / np.sqrt(np.float32(HD))

    in_maps = []
    bias_flags = None
    for c in range(NC):
        hg = [c * HL + t for t in range(HL)]
        qcols = np.concatenate([np.arange(h * HD, (h + 1) * HD) for h in hg])
        kcols = D + qcols
        vcols = 2 * D + qcols

        wq = Wattn[:nl][:, :, qcols] * qs
        wk = Wattn[:nl][:, :, kcols]
        wv = Wattn[:nl][:, :, vcols]
        wqkv = np.concatenate([wq, wk, wv], axis=2)  # [nl, D, 384]
        wqkv = ln1_g[:nl][:, :, None] * wqkv
        csqkv = wqkv.sum(axis=1, keepdims=True)  # [nl, 1, 384]
        bq = battn[:nl][:, qcols] * qs
        bk = battn[:nl][:, kcols]
        bv = battn[:nl][:, vcols]
        bqkv = np.concatenate([bq, bk, bv], axis=1)[:, None, :]
        bqkv = bqkv + np.einsum("ld,ldm->lm", ln1_b[:nl],
                                np.concatenate([Wattn[:nl][:, :, qcols] * qs,
                                                Wattn[:nl][:, :, kcols],
                                                Wattn[:nl][:, :, vcols]],
                                               axis=2))[:, None, :]

        rows = qcols  # proj rows for these heads
        wproj_c = np.ascontiguousarray(Wproj[:nl][:, rows, :])
        bproj_c = (bproj[:nl] / NC)[:, None, :]

        wfc_c = ln2_g[:nl][:, :, None] * Wfc[:nl][:, :,
                                                  c * DFS:(c + 1) * DFS]
        csfc_c = wfc_c.sum(axis=1, keepdims=True)
        bfc_c = (bfc[:nl][:, c * DFS:(c + 1) * DFS][:, None, :]
                 + np.einsum("ld,ldm->lm", ln2_b[:nl],
                             Wfc[:nl][:, :, c * DFS:(c + 1) * DFS])[:, None, :])
        wfc2_c = np.ascontiguousarray(Wfc2[:nl][:, c * DFS:(c + 1) * DFS, :])
        bfc2_c = (bfc2[:nl] / NC)[:, None, :]

        kivi = np.stack([
            np.ascontiguousarray(ki[:, :, qcols].transpose(0, 2, 1)),
            np.ascontiguousarray(vi[:, :, qcols].transpose(0, 2, 1)),
        ], axis=1)  # [nl, 2, 128, B]

        v0 = c * VSH
        v1 = min(V, v0 + VSH)
        wt_rows = wte[v0:v1] * lnf_g[None, :]  # [real, D]
        wteT_c = np.zeros((D, VS), np.float32)
        wteT_c[:, : v1 - v0] = wt_rows.T
        blm_row = lnf_b @ wte[v0:v1].T  # [real]
        blm_c = np.zeros((1, VS), np.float32)
        blm_c[0, : v1 - v0] = blm_row

        m = {
            "h0T": h0T, "wattn": np.ascontiguousarray(wqkv),
            "csqkv": np.ascontiguousarray(csqkv),
            "wproj": wproj_c,
            "wfc": np.ascontiguousarray(wfc_c),
            "csfc": np.ascontiguousarray(csfc_c),
            "wfc2": wfc2_c, "kivi": kivi, "mask": mask, "ident": ident,
            "wteT": wteT_c, "cones": cones,
        }
        m["_bqkv"] = np.ascontiguousarray(bqkv)
        m["_bproj"] = np.ascontiguousarray(bproj_c)
        m["_bfc"] = np.ascontiguousarray(bfc_c)
        m["_bfc2"] = np.ascontiguousarray(bfc2_c)
        m["_blm"] = blm_c
        in_maps.append(m)
    names = ("bqkv", "bproj", "bfc", "bfc2", "blm")
    bias_flags = tuple(
        bool(any(np.any(m["_" + n]) for m in in_maps)) for n in names)
    for m in in_maps:
        for n, flag in zip(names, bias_flags):
            arr = m.pop("_" + n)
            if flag:
                m[n] = arr
    return in_maps, nl, n_masks, bias_flags


_LAST_RESULTS = {}


def kernel(**inputs):
    in_maps, nl, n_masks, bias_flags = _prep(inputs)
    nc = _build(nl, n_masks, *bias_flags)
    trace = bool(int(os.environ.get("BASS_KERNEL_TRACE", "0")))
    res = bass_utils.run_bass_kernel_spmd(
        nc, in_maps, core_ids=list(range(NC)), trace=trace)
    _LAST_RESULTS["res"] = res
    logits = np.empty((T, V), np.float32)
    for c in range(NC):
        v0 = c * VSH
        v1 = min(V, v0 + VSH)
        logits[:, v0:v1] = res.results[c]["logits"][:, : v1 - v0]
    return logits.reshape(B, S, V)
